# revision 7
# baseline (speedup 1.0000x reference)
"""ChebNet (K=3, 3 ChebConv layers + MLP readout) on 8 Trainium2 NeuronCores.

Graph-parallel sharding (per hint): nodes sharded 8 ways (12500/core, padded
to 12544); edges sharded by dst-owner core. Each of the 6 SpMM steps:
bulk dma_gather of x_scaled[src] rows (f32, 256B) from a replicated DRAM
table, aggregation via one-hot matmuls on the TensorEngine into PSUM in
transposed orientation ([64 feat, 128 dst]), Chebyshev recurrence + layer
linear fused per 128-node tile, AllGather exchange of updated scaled slices
between steps, AllReduce for the per-graph readout.

Host-side prep (graph preprocessing): degrees, per-(dst-tile, src-bucket)
edge grouping padded to a core-uniform SPMD structure, int16 gather index
packing, dst_rel one-hot vectors, per-graph one-hot.
"""
import numpy as np

N_NODES = 100000
N_GRAPHS = 32
D = 64
NC = 8
NPC = N_NODES // NC            # 12500
NPC_PAD = 12544                # 98 tiles of 128
NTILES = NPC_PAD // 128        # 98
TAB_ROWS = NC * NPC_PAD        # 100352
BUCK = 32768
NB = 4
SGT = 4                        # tiles per super-group (gather buffer granularity)
NSG = (NTILES + SGT - 1) // SGT
MAXI = 1024                    # max idxs per dma_gather call (SWDGE ring limit)


def _tab_row(n):
    return (n // NPC) * NPC_PAD + (n % NPC)


def _prep(h, src, dst, graph_ids):
    deg = np.bincount(dst, minlength=N_NODES).astype(np.float32)
    dinv = np.clip(deg, 1.0, None) ** -0.5

    tabrow = _tab_row(np.arange(N_NODES))
    x0s = np.zeros((TAB_ROWS, D), np.float32)
    x0s[tabrow] = h * dinv[:, None]

    src_row = _tab_row(src)
    core_of = dst // NPC
    dloc = dst % NPC

    per_core = []
    counts = np.zeros((NC, NTILES, NB), np.int64)
    for c in range(NC):
        m = core_of == c
        es, ed = src_row[m], dloc[m]
        tile_id = ed >> 7
        buck = es >> 15
        order = np.lexsort((ed, buck, tile_id))
        es, ed, tile_id, buck = es[order], ed[order], tile_id[order], buck[order]
        key = tile_id * NB + buck
        cnt = np.bincount(key, minlength=NTILES * NB).reshape(NTILES, NB)
        counts[c] = cnt
        per_core.append((es, ed, cnt))

    cc = np.maximum(1, (counts.max(axis=0) + 127) // 128)  # [NTILES, NB]
    L = cc * 128
    total_chunks = int(cc.sum())

    # gather call list (uniform across cores): ordered (sg asc, b asc)
    calls = []
    idx_col_off = 0
    sg_buf_chunks = np.zeros((NSG, NB), np.int64)
    for sg in range(NSG):
        t0, t1 = sg * SGT, min((sg + 1) * SGT, NTILES)
        for b in range(NB):
            seg = int(L[t0:t1, b].sum())
            sg_buf_chunks[sg][b] = seg // 128
            off = 0
            while off < seg:
                ni = min(MAXI, seg - off)
                calls.append((sg, b, ni, off // 128, idx_col_off))
                idx_col_off += ni // 16
                off += ni
    idx_cols = idx_col_off

    idx_arrs, rel_arrs = [], []
    for c in range(NC):
        es, ed, cnt = per_core[c]
        starts = np.zeros(NTILES * NB + 1, np.int64)
        starts[1:] = cnt.reshape(-1).cumsum()
        rel = np.full((total_chunks, 128), -300, np.int32)
        stream = {b: [] for b in range(NB)}
        chunk_i = 0
        for t in range(NTILES):
            for b in range(NB):
                s0 = int(starts[t * NB + b])
                n = int(cnt[t][b])
                e = es[s0:s0 + n]
                d = ed[s0:s0 + n]
                pad = int(L[t][b] - n)
                i16 = np.concatenate([(e - BUCK * b).astype(np.int16),
                                      np.zeros(pad, np.int16)])
                stream[b].append(i16)
                dr = np.concatenate([(d - t * 128).astype(np.int32),
                                     np.full(pad, -300, np.int32)])
                nchunk = int(cc[t][b])
                rel[chunk_i:chunk_i + nchunk] = dr.reshape(nchunk, 128)
                chunk_i += nchunk
        streams = {b: np.concatenate(stream[b]) for b in range(NB)}
        pos = {b: 0 for b in range(NB)}
        packed = np.zeros((128, idx_cols), np.int16)
        for (sg, b, ni, bco, ico) in calls:
            a = streams[b][pos[b]:pos[b] + ni]
            pos[b] += ni
            packed[:, ico:ico + ni // 16] = np.tile(a.reshape(-1, 16).T, (8, 1))
        idx_arrs.append(packed)
        rel_arrs.append(np.ascontiguousarray(rel.T))

    cnt_g = np.bincount(graph_ids, minlength=N_GRAPHS).astype(np.float32)
    cnt_inv = (1.0 / np.clip(cnt_g, 1.0, None)).astype(np.float32)
    onehotG, dinvb = [], []
    for c in range(NC):
        oh = np.zeros((NPC_PAD, N_GRAPHS), np.float32)
        oh[np.arange(NPC), graph_ids[c * NPC:(c + 1) * NPC]] = 1.0
        onehotG.append(oh)
        dv = np.zeros(NPC_PAD, np.float32)
        dv[:NPC] = dinv[c * NPC:(c + 1) * NPC]
        dinvb.append(np.broadcast_to(dv[None, :], (D, NPC_PAD)).copy())

    meta = dict(cc=cc, calls=calls, sg_buf_chunks=sg_buf_chunks,
                total_chunks=total_chunks, idx_cols=idx_cols)
    return meta, dict(x0s=x0s, idx=idx_arrs, rel=rel_arrs, dinvb=dinvb,
                      onehotG=onehotG, cnt_inv=cnt_inv)


def _build(meta):
    from concourse import bass, mybir, bacc, tile
    from concourse.masks import make_identity

    cc = meta["cc"]; calls = meta["calls"]
    sg_buf_chunks = meta["sg_buf_chunks"]
    total_chunks = meta["total_chunks"]; idx_cols = meta["idx_cols"]
    f32, bf16, i32, i16 = (mybir.dt.float32, mybir.dt.bfloat16,
                           mybir.dt.int32, mybir.dt.int16)
    AF = mybir.ActivationFunctionType
    OP = mybir.AluOpType

    rel_off = np.zeros(NTILES * NB, np.int64)
    rel_off[1:] = cc.reshape(-1).cumsum()[:-1]
    rel_off = rel_off.reshape(NTILES, NB)

    nc = bacc.Bacc("TRN2", target_bir_lowering=True, debug=False, num_devices=NC)

    t_tab0 = nc.dram_tensor("x0s", [TAB_ROWS, D], f32, kind="ExternalInput")
    t_idx = nc.dram_tensor("idx", [128, idx_cols], i16, kind="ExternalInput")
    t_rel = nc.dram_tensor("rel", [128, total_chunks], i32, kind="ExternalInput")
    t_dinvb = nc.dram_tensor("dinvb", [D, NPC_PAD], f32, kind="ExternalInput")
    t_h0T = nc.dram_tensor("h0T", [D, NPC_PAD], f32, kind="ExternalInput")
    t_ohg = nc.dram_tensor("ohg", [NPC_PAD, N_GRAPHS], f32, kind="ExternalInput")
    t_cntinv = nc.dram_tensor("cnt_inv", [N_GRAPHS, 1], f32, kind="ExternalInput")
    t_w = nc.dram_tensor("cheb_w", [9 * D, D], f32, kind="ExternalInput")
    t_b = nc.dram_tensor("cheb_b", [D, 3], f32, kind="ExternalInput")
    t_m0w = nc.dram_tensor("m0w", [D, 32], f32, kind="ExternalInput")
    t_m1w = nc.dram_tensor("m1w", [32, 16], f32, kind="ExternalInput")
    t_m2w = nc.dram_tensor("m2w", [16, 3], f32, kind="ExternalInput")
    t_m0b = nc.dram_tensor("m0b", [N_GRAPHS, 32], f32, kind="ExternalInput")
    t_m1b = nc.dram_tensor("m1b", [N_GRAPHS, 16], f32, kind="ExternalInput")
    t_m2b = nc.dram_tensor("m2b", [N_GRAPHS, 3], f32, kind="ExternalInput")
    t_out = nc.dram_tensor("out", [N_GRAPHS, 3], f32, kind="ExternalOutput")

    with tile.TileContext(nc) as tc:
        with (
            tc.tile_pool(name="const", bufs=1) as cp,
            tc.tile_pool(name="gbuf", bufs=2) as gp,
            tc.tile_pool(name="work", bufs=3) as wp,
            tc.tile_pool(name="psA", bufs=3, space="PSUM") as psA,
            tc.tile_pool(name="psHG", bufs=1, space="PSUM") as psHG,
            tc.tile_pool(name="psB", bufs=2, space="PSUM") as psB,
            tc.tile_pool(name="dram", bufs=1, space="DRAM") as dp,
        ):
            s_rel = cp.tile([128, total_chunks], i32)
            nc.sync.dma_start(out=s_rel[:], in_=t_rel[:, :])
            s_iota = cp.tile([128, 128], i32)
            nc.gpsimd.iota(s_iota[:], pattern=[[1, 128]], base=0, channel_multiplier=0)
            s_ident = cp.tile([128, 128], f32)
            make_identity(nc, s_ident[:])
            s_wtmp = cp.tile([D, 9 * D], bf16)
            for blk in range(9):
                nc.gpsimd.dma_start(out=s_wtmp[:, blk * D:(blk + 1) * D],
                                    in_=t_w[blk * D:(blk + 1) * D, :])
            s_bias = cp.tile([D, 3], f32)
            nc.sync.dma_start(out=s_bias[:], in_=t_b[:, :])
            s_cntinv = cp.tile([N_GRAPHS, 1], f32)
            nc.sync.dma_start(out=s_cntinv[:], in_=t_cntinv[:, :])
            s_m0w = cp.tile([D, 32], f32); nc.sync.dma_start(out=s_m0w[:], in_=t_m0w[:, :])
            s_m1w = cp.tile([32, 16], f32); nc.sync.dma_start(out=s_m1w[:], in_=t_m1w[:, :])
            s_m2w = cp.tile([16, 3], f32); nc.sync.dma_start(out=s_m2w[:], in_=t_m2w[:, :])
            s_m0b = cp.tile([N_GRAPHS, 32], f32); nc.sync.dma_start(out=s_m0b[:], in_=t_m0b[:, :])
            s_m1b = cp.tile([N_GRAPHS, 16], f32); nc.sync.dma_start(out=s_m1b[:], in_=t_m1b[:, :])
            s_m2b = cp.tile([N_GRAPHS, 3], f32); nc.sync.dma_start(out=s_m2b[:], in_=t_m2b[:, :])

            s_x0T = cp.tile([D, NPC_PAD], bf16, name="x0T")
            s_x1T = cp.tile([D, NPC_PAD], bf16, name="x1T")
            s_x2T = cp.tile([D, NPC_PAD], bf16, name="x2T")
            nc.gpsimd.dma_start(out=s_x0T[:], in_=t_h0T[:, :])  # f32 -> bf16 cast

            d_slice = dp.tile([NPC_PAD, D], f32, name="bounce_in")
            d_tabs = [dp.tile([TAB_ROWS, D], f32, name=f"bounce_tab{i}",
                              addr_space="Shared") for i in range(5)]
            d_hg_in = dp.tile([N_GRAPHS, D], f32, name="hg_in")
            d_hg_out = dp.tile([N_GRAPHS, D], f32, name="hg_out", addr_space="Shared")

            def dv_tile(t, step):
                dv = wp.tile([D, 128], f32, tag="dv", name=f"dv_{step}_{t}")
                nc.sync.dma_start(out=dv[:], in_=t_dinvb[:, t * 128:(t + 1) * 128])
                return dv

            def spmm(table_ap, step):
                psum_tiles = {}
                callptr = 0
                for sg in range(NSG):
                    t0s, t1s = sg * SGT, min((sg + 1) * SGT, NTILES)
                    bufs = {}
                    for b in range(NB):
                        nch = int(sg_buf_chunks[sg][b])
                        buf = gp.tile([128, nch * D], f32, tag=f"gb{b}",
                                      name=f"g_{step}_{sg}_{b}")
                        bufbf = gp.tile([128, nch * D], bf16, tag=f"gc{b}",
                                        name=f"gc_{step}_{sg}_{b}")
                        bufs[b] = bufbf
                        while (callptr < len(calls) and calls[callptr][0] == sg
                               and calls[callptr][1] == b):
                            _, _, ni, bco, ico = calls[callptr]
                            six = wp.tile([128, ni // 16], i16, tag="idx",
                                          name=f"ix_{step}_{callptr}")
                            nc.sync.dma_start(out=six[:], in_=t_idx[:, ico:ico + ni // 16])
                            nc.gpsimd.dma_gather(
                                out_ap=buf[:, bco * D:(bco + ni // 128) * D].rearrange(
                                    "p (c d) -> p c d", d=D),
                                in_ap=table_ap[b * BUCK:min((b + 1) * BUCK, TAB_ROWS), :],
                                idxs_ap=six[:],
                                num_idxs=ni, num_idxs_reg=ni, elem_size=D)
                            nc.scalar.activation(
                                out=bufbf[:, bco * D:(bco + ni // 128) * D],
                                in_=buf[:, bco * D:(bco + ni // 128) * D],
                                func=AF.Copy)
                            callptr += 1
                    for t in range(t0s, t1s):
                        ps = psA.tile([D, 128], f32, tag="agg", name=f"agg_{step}_{t}")
                        psum_tiles[t] = ps
                        nmm = int(cc[t].sum())
                        done = 0
                        for b in range(NB):
                            base_chunk = int(cc[t0s:t, b].sum())
                            for k in range(int(cc[t][b])):
                                col = (base_chunk + k) * D
                                relcol = int(rel_off[t][b]) + k
                                oh = wp.tile([128, 128], bf16, tag="oh",
                                             name=f"oh_{step}_{t}_{b}_{k}")
                                nc.vector.tensor_tensor(
                                    out=oh[:],
                                    in0=s_rel[:, relcol:relcol + 1].to_broadcast([128, 128]),
                                    in1=s_iota[:], op=OP.is_equal)
                                done += 1
                                nc.tensor.matmul(out=ps[:],
                                                 lhsT=bufs[b][:, col:col + D],
                                                 rhs=oh[:],
                                                 start=(done == 1), stop=(done == nmm))
                return psum_tiles

            def exchange(i):
                nc.gpsimd.collective_compute(
                    "AllGather", OP.bypass, replica_groups=[list(range(NC))],
                    ins=[d_slice[:].opt()], outs=[d_tabs[i][:].opt()])

            def emit_scaled_rows(srcT_ap, t, dv, tagp):
                sc = wp.tile([D, 128], f32, tag="scl", name=f"sc_{tagp}_{t}")
                nc.vector.tensor_tensor(out=sc[:], in0=srcT_ap, in1=dv[:],
                                        op=OP.mult)
                pt = psB.tile([128, D], f32, tag="tr", name=f"tr_{tagp}_{t}")
                nc.tensor.transpose(out=pt[:], in_=sc[:], identity=s_ident[:64, :64])
                rows = wp.tile([128, D], f32, tag="rows", name=f"rw_{tagp}_{t}")
                nc.vector.tensor_copy(out=rows[:], in_=pt[:])
                nc.sync.dma_start(out=d_slice[t * 128:(t + 1) * 128, :], in_=rows[:])

            hg_ps = None
            for layer in range(3):
                tabA = t_tab0[:, :] if layer == 0 else d_tabs[2 * layer - 1][:]
                ps_a = spmm(tabA, step=2 * layer)
                for t in range(NTILES):
                    dv = dv_tile(t, f"a{layer}")
                    x1T = wp.tile([D, 128], f32, tag="x1w", name=f"x1_{layer}_{t}")
                    nc.vector.scalar_tensor_tensor(
                        out=x1T[:], in0=ps_a[t][:], scalar=-1.0, in1=dv[:],
                        op0=OP.mult, op1=OP.mult)
                    nc.vector.tensor_copy(out=s_x1T[:, t * 128:(t + 1) * 128], in_=x1T[:])
                    emit_scaled_rows(x1T[:], t, dv, f"a{layer}")
                exchange(2 * layer)
                ps_b = spmm(d_tabs[2 * layer][:], step=2 * layer + 1)
                for t in range(NTILES):
                    dv = dv_tile(t, f"b{layer}")
                    tmp = wp.tile([D, 128], f32, tag="x2w", name=f"x2_{layer}_{t}")
                    nc.vector.scalar_tensor_tensor(
                        out=tmp[:], in0=ps_b[t][:], scalar=-2.0, in1=dv[:],
                        op0=OP.mult, op1=OP.mult)
                    x0f = wp.tile([D, 128], f32, tag="x0f", name=f"x0f_{layer}_{t}")
                    nc.vector.tensor_copy(out=x0f[:], in_=s_x0T[:, t * 128:(t + 1) * 128])
                    nc.vector.tensor_tensor(
                        out=s_x2T[:, t * 128:(t + 1) * 128], in0=tmp[:], in1=x0f[:],
                        op=OP.subtract)
                terms = [s_x0T, s_x1T, s_x2T]
                last = layer == 2
                for t in range(NTILES):
                    po = psB.tile([D, 128], f32, tag="lin", name=f"lin_{layer}_{t}")
                    for j in range(3):
                        nc.tensor.matmul(
                            out=po[:],
                            lhsT=s_wtmp[:, (3 * layer + j) * D:(3 * layer + j + 1) * D],
                            rhs=terms[j][:, t * 128:(t + 1) * 128],
                            start=(j == 0), stop=(j == 2))
                    oT = wp.tile([D, 128], f32, tag="oT", name=f"o_{layer}_{t}")
                    if last:
                        nc.vector.tensor_tensor(
                            out=oT[:], in0=po[:],
                            in1=s_bias[:, layer:layer + 1].to_broadcast([D, 128]),
                            op=OP.add)
                    else:
                        nc.scalar.activation(out=oT[:], in_=po[:], func=AF.Relu,
                                             bias=s_bias[:, layer:layer + 1])
                    if not last:
                        nc.vector.tensor_copy(out=s_x0T[:, t * 128:(t + 1) * 128], in_=oT[:])
                        dv = dv_tile(t, f"o{layer}")
                        emit_scaled_rows(oT[:], t, dv, f"o{layer}")
                    else:
                        pt = psB.tile([128, D], f32, tag="tr", name=f"tr3_{t}")
                        nc.tensor.transpose(out=pt[:], in_=oT[:], identity=s_ident[:64, :64])
                        rows = wp.tile([128, D], f32, tag="rows", name=f"rw3_{t}")
                        nc.vector.tensor_copy(out=rows[:], in_=pt[:])
                        ohg = wp.tile([128, N_GRAPHS], f32, tag="ohg", name=f"ohg_{t}")
                        nc.sync.dma_start(out=ohg[:], in_=t_ohg[t * 128:(t + 1) * 128, :])
                        if hg_ps is None:
                            hg_ps = psHG.tile([N_GRAPHS, D], f32, tag="hg", name="hg_ps")
                        nc.tensor.matmul(out=hg_ps[:], lhsT=ohg[:], rhs=rows[:],
                                         start=(t == 0), stop=(t == NTILES - 1))
                if not last:
                    exchange(2 * layer + 1)

            hg_sb = cp.tile([N_GRAPHS, D], f32)
            nc.vector.tensor_copy(out=hg_sb[:], in_=hg_ps[:])
            nc.sync.dma_start(out=d_hg_in[:, :], in_=hg_sb[:])
            nc.gpsimd.collective_compute(
                "AllReduce", OP.add, replica_groups=[list(range(NC))],
                ins=[d_hg_in[:].opt()], outs=[d_hg_out[:].opt()])
            hg2 = cp.tile([N_GRAPHS, D], f32)
            nc.sync.dma_start(out=hg2[:], in_=d_hg_out[:, :])
            hg3 = cp.tile([N_GRAPHS, D], f32)
            nc.vector.tensor_tensor(out=hg3[:], in0=hg2[:],
                                    in1=s_cntinv[:, 0:1].to_broadcast([N_GRAPHS, D]),
                                    op=OP.mult)
            pT = psB.tile([D, N_GRAPHS], f32, tag="tr", name="hgT_ps")
            nc.tensor.transpose(out=pT[:], in_=hg3[:], identity=s_ident[:32, :32])
            hgT = cp.tile([D, N_GRAPHS], f32)
            nc.vector.tensor_copy(out=hgT[:], in_=pT[:])
            p1 = psB.tile([N_GRAPHS, 32], f32, tag="lin", name="m1_ps")
            nc.tensor.matmul(out=p1[:], lhsT=hgT[:], rhs=s_m0w[:], start=True, stop=True)
            a1 = cp.tile([N_GRAPHS, 32], f32)
            nc.vector.tensor_tensor(out=a1[:], in0=p1[:], in1=s_m0b[:], op=OP.add)
            r1 = cp.tile([N_GRAPHS, 32], f32)
            nc.scalar.activation(out=r1[:], in_=a1[:], func=AF.Relu)
            pT1 = psB.tile([32, N_GRAPHS], f32, tag="tr", name="r1T_ps")
            nc.tensor.transpose(out=pT1[:], in_=r1[:], identity=s_ident[:32, :32])
            r1T = cp.tile([32, N_GRAPHS], f32)
            nc.vector.tensor_copy(out=r1T[:], in_=pT1[:])
            p2 = psB.tile([N_GRAPHS, 16], f32, tag="lin", name="m2_ps")
            nc.tensor.matmul(out=p2[:], lhsT=r1T[:], rhs=s_m1w[:], start=True, stop=True)
            a2 = cp.tile([N_GRAPHS, 16], f32)
            nc.vector.tensor_tensor(out=a2[:], in0=p2[:], in1=s_m1b[:], op=OP.add)
            r2 = cp.tile([N_GRAPHS, 16], f32)
            nc.scalar.activation(out=r2[:], in_=a2[:], func=AF.Relu)
            pT2 = psB.tile([16, N_GRAPHS], f32, tag="tr", name="r2T_ps")
            nc.tensor.transpose(out=pT2[:], in_=r2[:], identity=s_ident[:32, :32])
            r2T = cp.tile([16, N_GRAPHS], f32)
            nc.vector.tensor_copy(out=r2T[:], in_=pT2[:])
            p3 = psB.tile([N_GRAPHS, 3], f32, tag="lin", name="m3_ps")
            nc.tensor.matmul(out=p3[:], lhsT=r2T[:], rhs=s_m2w[:], start=True, stop=True)
            a3 = cp.tile([N_GRAPHS, 3], f32)
            nc.vector.tensor_tensor(out=a3[:], in0=p3[:], in1=s_m2b[:], op=OP.add)
            nc.sync.dma_start(out=t_out[:, :], in_=a3[:])

    nc.compile()
    return nc


def _make_runner(nc, n_cores):
    import jax
    from jax.sharding import Mesh, PartitionSpec, NamedSharding
    from jax.experimental.shard_map import shard_map
    from concourse import mybir
    from concourse.bass2jax import (_bass_exec_p, install_neuronx_cc_hook,
                                    partition_id_tensor)

    install_neuronx_cc_hook()
    partition_name = nc.partition_id_tensor.name if nc.partition_id_tensor else None
    dbg_name = nc.dbg_addr.name if nc.dbg_addr is not None else None
    in_names, out_names, out_avals, zero_outs = [], [], [], []
    for alloc in nc.m.functions[0].allocations:
        if not isinstance(alloc, mybir.MemoryLocationSet):
            continue
        name = alloc.memorylocations[0].name
        if alloc.kind == "ExternalInput":
            if name != partition_name and name != dbg_name:
                in_names.append(name)
        elif alloc.kind == "ExternalOutput":
            out_names.append(name)
            out_avals.append(jax.core.ShapedArray(
                tuple(alloc.tensor_shape), mybir.dt.np(alloc.dtype)))
            zero_outs.append(np.zeros(tuple(alloc.tensor_shape),
                                      mybir.dt.np(alloc.dtype)))

    n_params = len(in_names)
    all_in_names = list(in_names) + list(out_names)
    if dbg_name is not None:
        all_in_names.append(dbg_name)
    if partition_name is not None:
        all_in_names.append(partition_name)

    def _body(*args):
        operands = list(args)
        if dbg_name is not None:
            operands.append(jax.numpy.zeros((1, 2), jax.numpy.uint32))
        if partition_name is not None:
            operands.append(partition_id_tensor())
        return tuple(_bass_exec_p.bind(
            *operands, out_avals=tuple(out_avals), in_names=tuple(all_in_names),
            out_names=tuple(out_names), lowering_input_output_aliases=(),
            sim_require_finite=True, sim_require_nnan=True, nc=nc))

    donate = tuple(range(n_params, n_params + len(out_avals)))
    devices = jax.devices()[:n_cores]
    mesh = Mesh(np.asarray(devices), ("core",))
    fn = jax.jit(
        shard_map(_body, mesh=mesh,
                  in_specs=(PartitionSpec("core"),) * (n_params + len(out_avals)),
                  out_specs=(PartitionSpec("core"),) * len(out_names),
                  check_rep=False),
        donate_argnums=donate, keep_unused=True)

    dev_cache = {}

    def run_multi(in_maps, cache_key=None):
        if cache_key is not None and cache_key in dev_cache:
            concat_in = dev_cache[cache_key]
        else:
            concat_in = [
                np.concatenate([np.asarray(m[name]) for m in in_maps], axis=0)
                for name in in_names]
            if cache_key is not None:
                sh = NamedSharding(mesh, PartitionSpec("core"))
                concat_in = [jax.device_put(a, sh) for a in concat_in]
                jax.block_until_ready(concat_in)
                dev_cache.clear()
                dev_cache[cache_key] = concat_in
        concat_zeros = [np.zeros((n_cores * z.shape[0], *z.shape[1:]), z.dtype)
                        for z in zero_outs]
        out_arrs = fn(*concat_in, *concat_zeros)
        jax.block_until_ready(out_arrs)
        return [{name: np.asarray(out_arrs[i]).reshape(
                    n_cores, *out_avals[i].shape)[c]
                 for i, name in enumerate(out_names)}
                for c in range(n_cores)]

    return run_multi


_compiled = {}


def kernel(h, src, dst, graph_ids, W0, b0, W1, b1, W2, b2,
           M0w, M0b, M1w, M1b, M2w, M2b):
    h = np.asarray(h, np.float32)
    src = np.asarray(src, np.int32)
    dst = np.asarray(dst, np.int32)
    graph_ids = np.asarray(graph_ids, np.int32)

    meta, data = _prep(h, src, dst, graph_ids)

    key = "k"
    if key not in _compiled:
        nc = _build(meta)
        _compiled[key] = (_make_runner(nc, NC), meta)
    run, _ = _compiled[key]

    cheb_w = np.concatenate([np.asarray(w, np.float32) for w in (W0, W1, W2)], axis=0)
    cheb_b = np.stack([np.asarray(b, np.float32) for b in (b0, b1, b2)], axis=1)  # [64, 3]

    in_maps = []
    for c in range(NC):
        h0T = np.zeros((D, NPC_PAD), np.float32)
        h0T[:, :NPC] = h[c * NPC:(c + 1) * NPC].T
        in_maps.append({
            "x0s": data["x0s"], "idx": data["idx"][c], "rel": data["rel"][c],
            "dinvb": data["dinvb"][c], "h0T": h0T, "ohg": data["onehotG"][c],
            "cnt_inv": data["cnt_inv"].reshape(N_GRAPHS, 1),
            "cheb_w": cheb_w, "cheb_b": cheb_b,
            "m0w": np.asarray(M0w, np.float32),
            "m1w": np.asarray(M1w, np.float32),
            "m2w": np.asarray(M2w, np.float32),
            "m0b": np.broadcast_to(np.asarray(M0b, np.float32)[None, :], (N_GRAPHS, 32)).copy(),
            "m1b": np.broadcast_to(np.asarray(M1b, np.float32)[None, :], (N_GRAPHS, 16)).copy(),
            "m2b": np.broadcast_to(np.asarray(M2b, np.float32)[None, :], (N_GRAPHS, 3)).copy(),
        })

    res = run(in_maps)
    kernel._last_run = (run, in_maps)
    return res[0]["out"]
